# revision 14
# baseline (speedup 1.0000x reference)
"""Multi-head attention TRN2 kernel (8 NeuronCores).

Sharding: core c = (batch b = c//4, head-group g = c%4) -> 4 heads per core.
Device computes, per core:
  qT/kT  [256f, S]   (feature-major projections, bias via ACT)
  v      [S, 260]    (token-major projection, 65-stride per-head layout with
                      a ones column per head for row-sums)
  scoresT[k, q] = kT^T-blocks @ qT  (+ additive fp16 mask via identity matmul)
  expT = exp(scoresT/8)  (ACT)
  ctx_augT[65, q] = v_aug^T @ expT  (row 64 = sums of exp over k)
  r = 1/sums; broadcast via PE outer product; normalize expT -> attnT (DMA out)
  normalize ctx -> ctxT; out_partial = ctxT^T @ wo  (DMA out)
Host: sums out partials over the 4 cores of each batch (+bo), and returns
attn as a zero-copy transpose view of the gathered attnT.
"""

import sys

if "/opt/trn_rl_repo" not in sys.path:
    sys.path.insert(0, "/opt/trn_rl_repo")

import numpy as np

B, S, DM, H, DK, DV = 2, 2048, 1024, 16, 64, 64
HPC = 4            # heads per core
FPC = HPC * DK     # features per core (256)
N_CORES = 8
QC = 512           # q-chunk (matmul moving free dim)
NEG = -16384.0     # additive mask constant; exact in fp16, exp() underflows to 0

_CACHE = {}


def build_program(s=S, reps=1, mm_dtype="float32r"):
    import concourse.bass as bass
    import concourse.tile as tile
    from concourse import bacc, mybir
    from contextlib import ExitStack

    f32 = mybir.dt.float32
    f16 = mybir.dt.float16
    mmdt = getattr(mybir.dt, mm_dtype)
    AF = mybir.ActivationFunctionType

    n_tok_tiles = s // 128          # token tiles (16)
    n_qc = s // QC                  # q chunks (4)
    n_kt = s // 128                 # k tiles (16)
    n_dm = DM // 128                # dm chunks (8)

    nc = bacc.Bacc("TRN2", target_bir_lowering=False, debug=False)

    qt_d = nc.dram_tensor("qt", [DM, s], mmdt, kind="ExternalInput").ap()
    kt_d = nc.dram_tensor("kt", [DM, s], mmdt, kind="ExternalInput").ap()
    vt_d = nc.dram_tensor("vt", [DM, s], mmdt, kind="ExternalInput").ap()
    maskt_d = nc.dram_tensor("maskt", [s, s], f16, kind="ExternalInput").ap()
    wq_d = nc.dram_tensor("wq", [DM, FPC], mmdt, kind="ExternalInput").ap()
    wk_d = nc.dram_tensor("wk", [DM, FPC], mmdt, kind="ExternalInput").ap()
    wv_d = nc.dram_tensor("wv", [DM, FPC], mmdt, kind="ExternalInput").ap()
    wo_d = nc.dram_tensor("wo", [FPC, DM], mmdt, kind="ExternalInput").ap()
    bq_d = nc.dram_tensor("bq", [128, 2], f32, kind="ExternalInput").ap()
    bk_d = nc.dram_tensor("bk", [128, 2], f32, kind="ExternalInput").ap()
    bv_d = nc.dram_tensor("bv", [1, FPC], mmdt, kind="ExternalInput").ap()
    ident_d = nc.dram_tensor("ident", [128, 128], f16, kind="ExternalInput").ap()
    ones_d = nc.dram_tensor("onesd", [128, 128], mmdt, kind="ExternalInput").ap()
    attn_d = nc.dram_tensor("attnt", [HPC, s, s], f32, kind="ExternalOutput").ap()
    out_d = nc.dram_tensor("outp", [s, DM], f32, kind="ExternalOutput").ap()

    def mm(out, lhsT, rhs, start, stop):
        nc.tensor.matmul(out, lhsT, rhs, start=start, stop=stop)

    with tile.TileContext(nc) as tc, ExitStack() as ctx:
        consts = ctx.enter_context(tc.tile_pool(name="consts", bufs=1))
        persist = ctx.enter_context(tc.tile_pool(name="persist", bufs=1))

        ident_sb = consts.tile([128, 128], f16)
        nc.sync.dma_start(ident_sb[:], ident_d[:])
        bq_sb = consts.tile([128, 2], f32)
        nc.sync.dma_start(bq_sb[:], bq_d[:])
        bk_sb = consts.tile([128, 2], f32)
        nc.sync.dma_start(bk_sb[:], bk_d[:])
        bv_sb = consts.tile([1, FPC], mmdt)
        nc.sync.dma_start(bv_sb[:], bv_d[:])
        ones_sb = consts.tile([128, 128], mmdt)
        nc.sync.dma_start(ones_sb[:], ones_d[:])

        wo_sb = consts.tile([128, 2, DM], mmdt)
        for c in range(2):
            nc.sync.dma_start(wo_sb[:, c, :], wo_d[c * 128:(c + 1) * 128, :])

        # persistent activations
        qT_sb = persist.tile([128, 2, s], mmdt)   # [f%128, f//128, tok]
        kT_sb = persist.tile([128, 2, s], mmdt)
        v_sb = persist.tile([128, n_tok_tiles, HPC * 65], mmdt)  # token-major + ones cols
        ctxT_sb = persist.tile([128, 2, s], mmdt)

        ones_col_src = ones_d[:, 0:n_tok_tiles].rearrange("p (x y) -> p x y", y=1)
        for h in range(HPC):
            nc.sync.dma_start(v_sb[:, :, h * 65 + 64:h * 65 + 65], ones_col_src)

        for rep in range(reps):
            # ---------------- projections ----------------
            with tc.tile_pool(name="wqkv", bufs=1) as wpool, \
                 tc.tile_pool(name="xstream", bufs=3) as xs, \
                 tc.tile_pool(name="proj_ps", bufs=4, space="PSUM") as pps:
                wq_sb = wpool.tile([128, n_dm, FPC], mmdt, tag="w")
                wk_sb = wpool.tile([128, n_dm, FPC], mmdt, tag="w2")
                wv_sb = wpool.tile([128, n_dm, FPC], mmdt, tag="w3")
                for c in range(n_dm):
                    nc.sync.dma_start(wq_sb[:, c, :], wq_d[c * 128:(c + 1) * 128, :])
                    nc.sync.dma_start(wk_sb[:, c, :], wk_d[c * 128:(c + 1) * 128, :])
                    nc.sync.dma_start(wv_sb[:, c, :], wv_d[c * 128:(c + 1) * 128, :])

                # feature-major q/k projections (streamed in 1024-wide chunks)
                tj_w = min(2 * QC, s)
                n_tj = s // tj_w
                n_sub = tj_w // QC
                for src_d, w_sb, dst, b_sb in ((qt_d, wq_sb, qT_sb, bq_sb),
                                               (kt_d, wk_sb, kT_sb, bk_sb)):
                    for tj in range(n_tj):
                        ps = [pps.tile([128, QC], f32, tag="pqk", name="pqk")
                              for _ in range(2 * n_sub)]
                        for c in range(n_dm):
                            xt = xs.tile([128, tj_w], mmdt, tag="x")
                            nc.sync.dma_start(
                                xt[:], src_d[c * 128:(c + 1) * 128,
                                             tj * tj_w:(tj + 1) * tj_w])
                            for sub in range(n_sub):
                                for ft in range(2):
                                    mm(ps[sub * 2 + ft][:],
                                       w_sb[:, c, ft * 128:(ft + 1) * 128],
                                       xt[:, sub * QC:(sub + 1) * QC],
                                       start=(c == 0), stop=(c == n_dm - 1))
                        for sub in range(n_sub):
                            ti = tj * n_sub + sub
                            for ft in range(2):
                                nc.scalar.activation(dst[:, ft, ti * QC:(ti + 1) * QC],
                                                     ps[sub * 2 + ft][:], AF.Identity,
                                                     bias=b_sb[:, ft:ft + 1])

                # token-major v projection (4 tok tiles per chunk)
                for ti in range(n_qc):
                    psv = [pps.tile([128, FPC], f32, tag="pv", name="pv") for _ in range(4)]
                    for c in range(n_dm):
                        xt = xs.tile([128, QC], mmdt, tag="x")
                        nc.sync.dma_start(
                            xt[:], vt_d[c * 128:(c + 1) * 128, ti * QC:(ti + 1) * QC])
                        for tt in range(4):
                            mm(psv[tt][:], xt[:, tt * 128:(tt + 1) * 128], wv_sb[:, c, :],
                               start=(c == 0), stop=False)
                    for tt in range(4):
                        # + bv (rank-1: ones[1,128] x bv[1,FPC])
                        mm(psv[tt][:], ones_sb[0:1, 0:128], bv_sb[0:1, :],
                           start=False, stop=True)
                        gt = ti * 4 + tt
                        dst = v_sb[:, gt, :].rearrange("p (h x) -> p h x", h=HPC)[:, :, 0:64]
                        src = psv[tt][:].rearrange("p (h x) -> p h x", h=HPC)
                        nc.scalar.activation(dst, src, AF.Copy)

            # ---------------- attention ----------------
            with tc.tile_pool(name="mask", bufs=2) as mpool, \
                 tc.tile_pool(name="exps", bufs=24) as epool, \
                 tc.tile_pool(name="attn_out", bufs=6) as apool, \
                 tc.tile_pool(name="rrep", bufs=3) as rpool, \
                 tc.tile_pool(name="score_ps", bufs=2, space="PSUM") as sps, \
                 tc.tile_pool(name="ctx_ps", bufs=2, space="PSUM") as cps, \
                 tc.tile_pool(name="r_ps", bufs=2, space="PSUM") as rps:
                for qc in range(n_qc):
                    # whole mask column-block in one DMA: [k, qc] -> [128, kt, q]
                    m_all = mpool.tile([128, n_kt, QC], f16, tag="m")
                    msrc = maskt_d[:, qc * QC:(qc + 1) * QC].rearrange(
                        "(i p) q -> p i q", p=128)
                    nc.sync.dma_start(m_all[:], msrc)
                    for h in range(HPC):
                        hp, hb = h // 2, (h % 2) * 64  # f-tile index, partition base
                        pc = cps.tile([128, QC], f32, tag="pc")
                        ets = []
                        for kt in range(n_kt):
                            ps = sps.tile([128, QC], f32, tag="ps")
                            # additive mask via identity matmul (fp16)
                            nc.tensor.matmul(ps[:], ident_sb[:], m_all[:, kt, :],
                                             start=True, stop=False)
                            mm(ps[:],
                               kT_sb[hb:hb + 64, hp, kt * 128:(kt + 1) * 128],
                               qT_sb[hb:hb + 64, hp, qc * QC:(qc + 1) * QC],
                               start=False, stop=True)
                            et = epool.tile([128, QC], mmdt, tag="e")
                            nc.scalar.activation(et[:], ps[:], AF.Exp, scale=0.125)
                            ets.append(et)
                            mm(pc[0:65, :], v_sb[:, kt, h * 65:h * 65 + 65], et[:],
                               start=(kt == 0), stop=(kt == n_kt - 1))
                        # r = 1/sums (row 64), broadcast to 128 partitions via PE
                        rr = rpool.tile([128, QC], mmdt, tag="rr")
                        with nc.allow_low_precision(reason="f32r operand for bcast matmul"):
                            nc.vector.reciprocal(rr[64:65, :], pc[64:65, :])
                        pr = rps.tile([128, QC], f32, tag="pr")
                        mm(pr[:], ones_sb[64:65, 0:128], rr[64:65, :],
                           start=True, stop=True)
                        rrep = rpool.tile([128, QC], f32, tag="rrep")
                        nc.scalar.activation(rrep[:], pr[:], AF.Copy)
                        # normalized ctx -> ctxT
                        nc.vector.tensor_mul(
                            ctxT_sb[hb:hb + 64, hp, qc * QC:(qc + 1) * QC],
                            pc[0:64, :], rrep[0:64, :])
                        # normalized attn tiles -> HBM (transposed layout)
                        for kt in range(n_kt):
                            at = apool.tile([128, QC], f32, tag="a")
                            nc.vector.tensor_mul(at[:], ets[kt][:], rrep[:])
                            dma_eng = nc.sync if kt % 2 == 0 else nc.gpsimd
                            dma_eng.dma_start(
                                attn_d[h, kt * 128:(kt + 1) * 128, qc * QC:(qc + 1) * QC],
                                at[:])

            # ---------------- output projection ----------------
            with tc.tile_pool(name="out_ps", bufs=4, space="PSUM") as ops, \
                 tc.tile_pool(name="out_sb", bufs=4) as osb:
                for tt in range(n_tok_tiles):
                    for dh in range(2):
                        po = ops.tile([128, QC], f32, tag="po")
                        for fc in range(2):
                            mm(po[:], ctxT_sb[:, fc, tt * 128:(tt + 1) * 128],
                               wo_sb[:, fc, dh * QC:(dh + 1) * QC],
                               start=(fc == 0), stop=(fc == 1))
                        ot = osb.tile([128, QC], f32, tag="o")
                        nc.scalar.activation(ot[:], po[:], AF.Copy)
                        nc.sync.dma_start(
                            out_d[tt * 128:(tt + 1) * 128, dh * QC:(dh + 1) * QC], ot[:])

    nc.compile()
    return nc


def _prep_in_maps(Q, K, V, mask, wq, bq, wk, bk, wv, bv, wo, s):
    f32 = np.float32
    QT = [np.ascontiguousarray(np.asarray(Q[b], f32).T) for b in range(B)]
    KT = [np.ascontiguousarray(np.asarray(K[b], f32).T) for b in range(B)]
    VT = [np.ascontiguousarray(np.asarray(V[b], f32).T) for b in range(B)]
    MT = [np.where(np.asarray(mask[b]).T, np.float16(NEG), np.float16(0))
          for b in range(B)]
    ident = np.eye(128, dtype=np.float16)
    wq, wk, wv, wo = (np.asarray(x, f32) for x in (wq, wk, wv, wo))
    bq, bk, bv = (np.asarray(x, f32) for x in (bq, bk, bv))
    in_maps = []
    for c in range(N_CORES):
        b, g = c // 4, c % 4
        fs = slice(g * FPC, (g + 1) * FPC)
        in_maps.append({
            "qt": QT[b], "kt": KT[b], "vt": VT[b], "maskt": MT[b],
            "wq": np.ascontiguousarray(wq[:, fs]),
            "wk": np.ascontiguousarray(wk[:, fs]),
            "wv": np.ascontiguousarray(wv[:, fs]),
            "wo": np.ascontiguousarray(wo[fs, :]),
            "bq": np.ascontiguousarray(bq[fs].reshape(2, 128).T),
            "bk": np.ascontiguousarray(bk[fs].reshape(2, 128).T),
            "bv": bv[fs].reshape(1, FPC).copy(),
            "ident": ident,
            "onesd": np.ones((128, 128), np.float32),
        })
    return in_maps


def _get_executor():
    """Build the program and a reusable jitted SPMD executor (compile once)."""
    if "exec" in _CACHE:
        return _CACHE["exec"]
    import jax
    import numpy as _np
    from jax.sharding import Mesh, PartitionSpec
    from jax.experimental.shard_map import shard_map
    from concourse import bass2jax, mybir

    nc = build_program()
    bass2jax.install_neuronx_cc_hook()

    partition_name = nc.partition_id_tensor.name if nc.partition_id_tensor else None
    in_names, out_names, out_avals, zero_shapes = [], [], [], []
    for alloc in nc.m.functions[0].allocations:
        if not isinstance(alloc, mybir.MemoryLocationSet):
            continue
        name = alloc.memorylocations[0].name
        if alloc.kind == "ExternalInput":
            if name != partition_name:
                in_names.append(name)
        elif alloc.kind == "ExternalOutput":
            shape = tuple(alloc.tensor_shape)
            dtype = mybir.dt.np(alloc.dtype)
            out_names.append(name)
            out_avals.append(jax.core.ShapedArray(shape, dtype))
            zero_shapes.append((shape, dtype))
    n_params = len(in_names)
    all_names = in_names + out_names
    if partition_name is not None:
        all_names = all_names + [partition_name]
    donate = tuple(range(n_params, n_params + len(out_names)))

    def _body(*args):
        operands = list(args)
        if partition_name is not None:
            operands.append(bass2jax.partition_id_tensor())
        return tuple(bass2jax._bass_exec_p.bind(
            *operands, out_avals=tuple(out_avals), in_names=tuple(all_names),
            out_names=tuple(out_names), lowering_input_output_aliases=(),
            sim_require_finite=True, sim_require_nnan=True, nc=nc))

    devices = jax.devices()[:N_CORES]
    mesh = Mesh(_np.asarray(devices), ("core",))
    specs = (PartitionSpec("core"),) * (n_params + len(out_names))
    sharded = jax.jit(
        shard_map(_body, mesh=mesh, in_specs=specs,
                  out_specs=(PartitionSpec("core"),) * len(out_names), check_rep=False),
        donate_argnums=donate, keep_unused=True)

    def run(in_maps):
        concat_in = [
            _np.concatenate([_np.asarray(in_maps[c][nm]) for c in range(N_CORES)], axis=0)
            for nm in in_names]
        concat_zeros = [_np.zeros((N_CORES * sh[0], *sh[1:]), dt)
                        for sh, dt in zero_shapes]
        out_arrs = sharded(*concat_in, *concat_zeros)
        return [
            {nm: _np.asarray(out_arrs[i]).reshape(N_CORES, *out_avals[i].shape)[c]
             for i, nm in enumerate(out_names)}
            for c in range(N_CORES)]

    _CACHE["exec"] = run
    return run


def kernel(Q, K, V, mask, wq, bq, wk, bk, wv, bv, wo, bo):
    run = _get_executor()
    in_maps = _prep_in_maps(Q, K, V, mask, wq, bq, wk, bk, wv, bv, wo, S)
    res = run(in_maps)

    out = np.zeros((B, S, DM), np.float32)
    attn_t = np.empty((B, H, S, S), np.float32)
    for c in range(N_CORES):
        b, g = c // 4, c % 4
        out[b] += res[c]["outp"]
        attn_t[b, g * HPC:(g + 1) * HPC] = res[c]["attnt"]
    out += np.asarray(bo, np.float32)
    attn = attn_t.transpose(0, 1, 3, 2)
    return out, attn


# revision 16
# speedup vs baseline: 54950.8667x; 54950.8667x over previous
"""Multi-head attention TRN2 kernel (8 NeuronCores).

Sharding: core c = (batch b = c//4, head-group g = c%4) -> 4 heads per core.
Device computes, per core:
  qT/kT  [256f, S]   (feature-major projections, bias via ACT)
  v      [S, 260]    (token-major projection, 65-stride per-head layout with
                      a ones column per head for row-sums)
  scoresT[k, q] = kT^T-blocks @ qT  (+ additive fp16 mask via identity matmul)
  expT = exp(scoresT/8)  (ACT)
  ctx_augT[65, q] = v_aug^T @ expT  (row 64 = sums of exp over k)
  r = 1/sums; broadcast via PE outer product; normalize expT -> attnT (DMA out)
  normalize ctx -> ctxT; out_partial = ctxT^T @ wo  (DMA out)
Host: sums out partials over the 4 cores of each batch (+bo), and returns
attn as a zero-copy transpose view of the gathered attnT.
"""

import sys

if "/opt/trn_rl_repo" not in sys.path:
    sys.path.insert(0, "/opt/trn_rl_repo")

import numpy as np

B, S, DM, H, DK, DV = 2, 2048, 1024, 16, 64, 64
HPC = 4            # heads per core
FPC = HPC * DK     # features per core (256)
N_CORES = 8
QC = 512           # q-chunk (matmul moving free dim)
NEG = -16384.0     # additive mask constant; exact in fp16, exp() underflows to 0

_CACHE = {}


def build_program(s=S, reps=1, mm_dtype="float32r"):
    import concourse.bass as bass
    import concourse.tile as tile
    from concourse import bacc, mybir
    from contextlib import ExitStack

    f32 = mybir.dt.float32
    f16 = mybir.dt.float16
    mmdt = getattr(mybir.dt, mm_dtype)
    AF = mybir.ActivationFunctionType

    n_tok_tiles = s // 128          # token tiles (16)
    n_qc = s // QC                  # q chunks (4)
    n_kt = s // 128                 # k tiles (16)
    n_dm = DM // 128                # dm chunks (8)

    nc = bacc.Bacc("TRN2", target_bir_lowering=False, debug=False)

    qt_d = nc.dram_tensor("qt", [DM, s], mmdt, kind="ExternalInput").ap()
    kt_d = nc.dram_tensor("kt", [DM, s], mmdt, kind="ExternalInput").ap()
    vt_d = nc.dram_tensor("vt", [DM, s], mmdt, kind="ExternalInput").ap()
    maskt_d = nc.dram_tensor("maskt", [s, s], f16, kind="ExternalInput").ap()
    wq_d = nc.dram_tensor("wq", [DM, FPC], mmdt, kind="ExternalInput").ap()
    wk_d = nc.dram_tensor("wk", [DM, FPC], mmdt, kind="ExternalInput").ap()
    wv_d = nc.dram_tensor("wv", [DM, FPC], mmdt, kind="ExternalInput").ap()
    wo_d = nc.dram_tensor("wo", [FPC, DM], mmdt, kind="ExternalInput").ap()
    bq_d = nc.dram_tensor("bq", [128, 2], f32, kind="ExternalInput").ap()
    bk_d = nc.dram_tensor("bk", [128, 2], f32, kind="ExternalInput").ap()
    bv_d = nc.dram_tensor("bv", [1, FPC], mmdt, kind="ExternalInput").ap()
    ident_d = nc.dram_tensor("ident", [128, 128], f16, kind="ExternalInput").ap()
    ones_d = nc.dram_tensor("onesd", [128, 128], mmdt, kind="ExternalInput").ap()
    attn_d = nc.dram_tensor("attnt", [HPC, s, s], f32, kind="ExternalOutput").ap()
    out_d = nc.dram_tensor("outp", [s, DM], f32, kind="ExternalOutput").ap()

    def mm(out, lhsT, rhs, start, stop):
        nc.tensor.matmul(out, lhsT, rhs, start=start, stop=stop)

    with tile.TileContext(nc) as tc, ExitStack() as ctx:
        consts = ctx.enter_context(tc.tile_pool(name="consts", bufs=1))
        persist = ctx.enter_context(tc.tile_pool(name="persist", bufs=1))

        ident_sb = consts.tile([128, 128], f16)
        nc.sync.dma_start(ident_sb[:], ident_d[:])
        bq_sb = consts.tile([128, 2], f32)
        nc.sync.dma_start(bq_sb[:], bq_d[:])
        bk_sb = consts.tile([128, 2], f32)
        nc.sync.dma_start(bk_sb[:], bk_d[:])
        bv_sb = consts.tile([1, FPC], mmdt)
        nc.sync.dma_start(bv_sb[:], bv_d[:])
        ones_sb = consts.tile([128, 128], mmdt)
        nc.sync.dma_start(ones_sb[:], ones_d[:])

        wo_sb = consts.tile([128, 2, DM], mmdt)
        for c in range(2):
            nc.sync.dma_start(wo_sb[:, c, :], wo_d[c * 128:(c + 1) * 128, :])

        # persistent activations
        qT_sb = persist.tile([128, 2, s], mmdt)   # [f%128, f//128, tok]
        kT_sb = persist.tile([128, 2, s], mmdt)
        v_sb = persist.tile([128, n_tok_tiles, HPC * 65], mmdt)  # token-major + ones cols
        ctxT_sb = persist.tile([128, 2, s], mmdt)

        ones_col_src = ones_d[:, 0:n_tok_tiles].rearrange("p (x y) -> p x y", y=1)
        for h in range(HPC):
            nc.sync.dma_start(v_sb[:, :, h * 65 + 64:h * 65 + 65], ones_col_src)

        for rep in range(reps):
            # ---------------- projections ----------------
            with tc.tile_pool(name="wqkv", bufs=1) as wpool, \
                 tc.tile_pool(name="xstream", bufs=3) as xs, \
                 tc.tile_pool(name="proj_ps", bufs=4, space="PSUM") as pps:
                wq_sb = wpool.tile([128, n_dm, FPC], mmdt, tag="w")
                wk_sb = wpool.tile([128, n_dm, FPC], mmdt, tag="w2")
                wv_sb = wpool.tile([128, n_dm, FPC], mmdt, tag="w3")
                for c in range(n_dm):
                    nc.sync.dma_start(wq_sb[:, c, :], wq_d[c * 128:(c + 1) * 128, :])
                    nc.sync.dma_start(wk_sb[:, c, :], wk_d[c * 128:(c + 1) * 128, :])
                    nc.sync.dma_start(wv_sb[:, c, :], wv_d[c * 128:(c + 1) * 128, :])

                # feature-major q/k projections (streamed in 1024-wide chunks)
                tj_w = min(2 * QC, s)
                n_tj = s // tj_w
                n_sub = tj_w // QC
                for src_d, w_sb, dst, b_sb in ((qt_d, wq_sb, qT_sb, bq_sb),
                                               (kt_d, wk_sb, kT_sb, bk_sb)):
                    for tj in range(n_tj):
                        ps = [pps.tile([128, QC], f32, tag="pqk", name="pqk")
                              for _ in range(2 * n_sub)]
                        for c in range(n_dm):
                            xt = xs.tile([128, tj_w], mmdt, tag="x")
                            nc.sync.dma_start(
                                xt[:], src_d[c * 128:(c + 1) * 128,
                                             tj * tj_w:(tj + 1) * tj_w])
                            for sub in range(n_sub):
                                for ft in range(2):
                                    mm(ps[sub * 2 + ft][:],
                                       w_sb[:, c, ft * 128:(ft + 1) * 128],
                                       xt[:, sub * QC:(sub + 1) * QC],
                                       start=(c == 0), stop=(c == n_dm - 1))
                        for sub in range(n_sub):
                            ti = tj * n_sub + sub
                            for ft in range(2):
                                nc.scalar.activation(dst[:, ft, ti * QC:(ti + 1) * QC],
                                                     ps[sub * 2 + ft][:], AF.Identity,
                                                     bias=b_sb[:, ft:ft + 1])

                # token-major v projection (4 tok tiles per chunk)
                for ti in range(n_qc):
                    psv = [pps.tile([128, FPC], f32, tag="pv", name="pv") for _ in range(4)]
                    for c in range(n_dm):
                        xt = xs.tile([128, QC], mmdt, tag="x")
                        nc.sync.dma_start(
                            xt[:], vt_d[c * 128:(c + 1) * 128, ti * QC:(ti + 1) * QC])
                        for tt in range(4):
                            mm(psv[tt][:], xt[:, tt * 128:(tt + 1) * 128], wv_sb[:, c, :],
                               start=(c == 0), stop=False)
                    for tt in range(4):
                        # + bv (rank-1: ones[1,128] x bv[1,FPC])
                        mm(psv[tt][:], ones_sb[0:1, 0:128], bv_sb[0:1, :],
                           start=False, stop=True)
                        gt = ti * 4 + tt
                        dst = v_sb[:, gt, :].rearrange("p (h x) -> p h x", h=HPC)[:, :, 0:64]
                        src = psv[tt][:].rearrange("p (h x) -> p h x", h=HPC)
                        nc.scalar.activation(dst, src, AF.Copy)

            # ---------------- attention ----------------
            with tc.tile_pool(name="mask", bufs=2) as mpool, \
                 tc.tile_pool(name="exps", bufs=24) as epool, \
                 tc.tile_pool(name="attn_out", bufs=6) as apool, \
                 tc.tile_pool(name="rrep", bufs=3) as rpool, \
                 tc.tile_pool(name="score_ps", bufs=2, space="PSUM") as sps, \
                 tc.tile_pool(name="ctx_ps", bufs=2, space="PSUM") as cps, \
                 tc.tile_pool(name="r_ps", bufs=2, space="PSUM") as rps:
                for qc in range(n_qc):
                    # whole mask column-block in one DMA: [k, qc] -> [128, kt, q]
                    m_all = mpool.tile([128, n_kt, QC], f16, tag="m")
                    msrc = maskt_d[:, qc * QC:(qc + 1) * QC].rearrange(
                        "(i p) q -> p i q", p=128)
                    nc.sync.dma_start(m_all[:], msrc)
                    for h in range(HPC):
                        hp, hb = h // 2, (h % 2) * 64  # f-tile index, partition base
                        pc = cps.tile([128, QC], f32, tag="pc")
                        ets = []
                        for kt in range(n_kt):
                            ps = sps.tile([128, QC], f32, tag="ps")
                            # additive mask via identity matmul (fp16)
                            nc.tensor.matmul(ps[:], ident_sb[:], m_all[:, kt, :],
                                             start=True, stop=False)
                            mm(ps[:],
                               kT_sb[hb:hb + 64, hp, kt * 128:(kt + 1) * 128],
                               qT_sb[hb:hb + 64, hp, qc * QC:(qc + 1) * QC],
                               start=False, stop=True)
                            et = epool.tile([128, QC], mmdt, tag="e")
                            nc.scalar.activation(et[:], ps[:], AF.Exp, scale=0.125)
                            ets.append(et)
                            mm(pc[0:65, :], v_sb[:, kt, h * 65:h * 65 + 65], et[:],
                               start=(kt == 0), stop=(kt == n_kt - 1))
                        # r = 1/sums (row 64), broadcast to 128 partitions via PE
                        rr = rpool.tile([128, QC], mmdt, tag="rr")
                        with nc.allow_low_precision(reason="f32r operand for bcast matmul"):
                            nc.vector.reciprocal(rr[64:65, :], pc[64:65, :])
                        pr = rps.tile([128, QC], f32, tag="pr")
                        mm(pr[:], ones_sb[64:65, 0:128], rr[64:65, :],
                           start=True, stop=True)
                        rrep = rpool.tile([128, QC], f32, tag="rrep")
                        nc.scalar.activation(rrep[:], pr[:], AF.Copy)
                        # normalized ctx -> ctxT
                        nc.vector.tensor_mul(
                            ctxT_sb[hb:hb + 64, hp, qc * QC:(qc + 1) * QC],
                            pc[0:64, :], rrep[0:64, :])
                        # normalized attn tiles -> HBM (transposed layout)
                        for kt in range(n_kt):
                            at = apool.tile([128, QC], f32, tag="a")
                            nc.vector.tensor_mul(at[:], ets[kt][:], rrep[:])
                            dma_eng = nc.sync if kt % 2 == 0 else nc.gpsimd
                            dma_eng.dma_start(
                                attn_d[h, kt * 128:(kt + 1) * 128, qc * QC:(qc + 1) * QC],
                                at[:])

            # ---------------- output projection ----------------
            with tc.tile_pool(name="out_ps", bufs=4, space="PSUM") as ops, \
                 tc.tile_pool(name="out_sb", bufs=4) as osb:
                for tt in range(n_tok_tiles):
                    for dh in range(2):
                        po = ops.tile([128, QC], f32, tag="po")
                        for fc in range(2):
                            mm(po[:], ctxT_sb[:, fc, tt * 128:(tt + 1) * 128],
                               wo_sb[:, fc, dh * QC:(dh + 1) * QC],
                               start=(fc == 0), stop=(fc == 1))
                        ot = osb.tile([128, QC], f32, tag="o")
                        nc.scalar.activation(ot[:], po[:], AF.Copy)
                        nc.sync.dma_start(
                            out_d[tt * 128:(tt + 1) * 128, dh * QC:(dh + 1) * QC], ot[:])

    nc.compile()
    return nc


def _prep_in_maps(Q, K, V, mask, wq, bq, wk, bk, wv, bv, wo, s):
    f32 = np.float32
    QT = [np.ascontiguousarray(np.asarray(Q[b], f32).T) for b in range(B)]
    KT = [np.ascontiguousarray(np.asarray(K[b], f32).T) for b in range(B)]
    VT = [np.ascontiguousarray(np.asarray(V[b], f32).T) for b in range(B)]
    MT = [np.where(np.asarray(mask[b]).T, np.float16(NEG), np.float16(0))
          for b in range(B)]
    ident = np.eye(128, dtype=np.float16)
    wq, wk, wv, wo = (np.asarray(x, f32) for x in (wq, wk, wv, wo))
    bq, bk, bv = (np.asarray(x, f32) for x in (bq, bk, bv))
    in_maps = []
    for c in range(N_CORES):
        b, g = c // 4, c % 4
        fs = slice(g * FPC, (g + 1) * FPC)
        in_maps.append({
            "qt": QT[b], "kt": KT[b], "vt": VT[b], "maskt": MT[b],
            "wq": np.ascontiguousarray(wq[:, fs]),
            "wk": np.ascontiguousarray(wk[:, fs]),
            "wv": np.ascontiguousarray(wv[:, fs]),
            "wo": np.ascontiguousarray(wo[fs, :]),
            "bq": np.ascontiguousarray(bq[fs].reshape(2, 128).T),
            "bk": np.ascontiguousarray(bk[fs].reshape(2, 128).T),
            "bv": bv[fs].reshape(1, FPC).copy(),
            "ident": ident,
            "onesd": np.ones((128, 128), np.float32),
        })
    return in_maps


def _get_executor():
    """Build the program and a reusable jitted SPMD executor (compile once)."""
    if "exec" in _CACHE:
        return _CACHE["exec"]
    import jax
    import numpy as _np
    from jax.sharding import Mesh, PartitionSpec
    from jax.experimental.shard_map import shard_map
    from concourse import bass2jax, mybir

    nc = build_program()
    bass2jax.install_neuronx_cc_hook()

    partition_name = nc.partition_id_tensor.name if nc.partition_id_tensor else None
    in_names, out_names, out_avals, zero_shapes = [], [], [], []
    for alloc in nc.m.functions[0].allocations:
        if not isinstance(alloc, mybir.MemoryLocationSet):
            continue
        name = alloc.memorylocations[0].name
        if alloc.kind == "ExternalInput":
            if name != partition_name:
                in_names.append(name)
        elif alloc.kind == "ExternalOutput":
            shape = tuple(alloc.tensor_shape)
            dtype = mybir.dt.np(alloc.dtype)
            out_names.append(name)
            out_avals.append(jax.core.ShapedArray(shape, dtype))
            zero_shapes.append((shape, dtype))
    n_params = len(in_names)
    all_names = in_names + out_names
    if partition_name is not None:
        all_names = all_names + [partition_name]
    donate = tuple(range(n_params, n_params + len(out_names)))

    def _body(*args):
        operands = list(args)
        if partition_name is not None:
            operands.append(bass2jax.partition_id_tensor())
        return tuple(bass2jax._bass_exec_p.bind(
            *operands, out_avals=tuple(out_avals), in_names=tuple(all_names),
            out_names=tuple(out_names), lowering_input_output_aliases=(),
            sim_require_finite=True, sim_require_nnan=True, nc=nc))

    devices = jax.devices()[:N_CORES]
    mesh = Mesh(_np.asarray(devices), ("core",))
    specs = (PartitionSpec("core"),) * (n_params + len(out_names))
    sharded = jax.jit(
        shard_map(_body, mesh=mesh, in_specs=specs,
                  out_specs=(PartitionSpec("core"),) * len(out_names), check_rep=False),
        donate_argnums=donate, keep_unused=True)

    def run(in_maps):
        concat_in = [
            _np.concatenate([_np.asarray(in_maps[c][nm]) for c in range(N_CORES)], axis=0)
            for nm in in_names]
        concat_zeros = [_np.zeros((N_CORES * sh[0], *sh[1:]), dt)
                        for sh, dt in zero_shapes]
        out_arrs = sharded(*concat_in, *concat_zeros)
        return [
            {nm: _np.asarray(out_arrs[i]).reshape(N_CORES, *out_avals[i].shape)[c]
             for i, nm in enumerate(out_names)}
            for c in range(N_CORES)]

    _CACHE["exec"] = run
    return run


def kernel(Q, K, V, mask, wq, bq, wk, bk, wv, bv, wo, bo):
    run = _get_executor()
    in_maps = _prep_in_maps(Q, K, V, mask, wq, bq, wk, bk, wv, bv, wo, S)
    res = run(in_maps)

    out = np.zeros((B, S, DM), np.float32)
    attn_t = np.empty((B, H, S, S), np.float32)
    for c in range(N_CORES):
        b, g = c // 4, c % 4
        out[b] += res[c]["outp"]
        attn_t[b, g * HPC:(g + 1) * HPC] = res[c]["attnt"]
    out += np.asarray(bo, np.float32)
    attn = attn_t.transpose(0, 1, 3, 2)
    return out, attn
